# revision 11
# baseline (speedup 1.0000x reference)
"""AffineCoupling TRN2 kernel (v7).

Computes, for z [4_000_000, 16] fp32:
    zl = z[:, :8]; zr = z[:, 8:]
    log_s = MLP_logs(zl); b = MLP_b(zl)        (5 layers, LeakyReLU(0.01) between)
    out = concat([zl, yr]), yr = exp(log_s) * zr + b

Strategy (pure data parallel over 8 NeuronCores, 524288 rows each):
 - 16 super-macros of 32768 rows. natbf [128, 4096] bf16 (256 rows/partition,
   nat[p, c*16+f] = row p*256+c) loaded by one SWDGE cast-DMA (fp32 HBM ->
   bf16 SBUF, 16KB contiguous HBM per partition) and stored back by one
   SWDGE cast-DMA (bf16 -> fp32). Whole pipeline is bf16 (zl passthrough
   in bf16: ~1.3e-3 rel err, tolerance 2e-2).
 - fwdT: 32 PE transpose-mode ops [128,128] -> bf16 PSUM -> DVE 2x copy
   -> x0 (feature-major: partition g*16+f, 8 groups of 16).
 - MLP: dual-branch block-diagonal bf16 lhsT; four column-quarters
   [128, 1024] ping-pong two PSUM tag slots so PE matmuls of quarter q+1
   overlap ACT Prelu of quarter q (Prelu+bias in one ACT op).
 - L5 packs e of quarter-pairs into full-lane tiles via M=64 col-tiled
   matmuls (e of q=2P -> partitions 0:64, q=2P+1 -> 64:128), so Exp and
   the b-branch tensor_scalar_add run at full 128-lane width.
 - backT: transpose-mode -> eT/bT bf16 PSUM; combine in place on natbf:
   zr = e*zr + b (two DVE TTs per quarter, u-batched strided APs).
 - Emission is software-pipelined: next super's fwd transposes ride the
   MLP layer slots, its L1 fills the Exp window, and backT/combine/store
   trail at the iteration end.
 - PSUM: T0(2) + T1(2) + x0ps(2) + eT(1) + bT(1) = 8 banks.
"""
import os
import sys

sys.path.insert(0, "/opt/trn_rl_repo")
if "/root/.axon_site/_ro/trn_rl_repo" not in sys.path:
    sys.path.append("/root/.axon_site/_ro/trn_rl_repo")

import numpy as np

import concourse.bacc as bacc
import concourse.bass as bass
import concourse.tile as tile
from concourse import mybir
from concourse.bass import _add_dep_helper
from concourse.bass_utils import run_bass_kernel_spmd

FP = mybir.dt.float32
BF = mybir.dt.bfloat16

N_CORES = 8
BATCH = 4_000_000
ROWS_PER_SUPER = 32_768            # [128, 4096] bf16 nat tile, 256 rows/part
SUPERS = 16
R = ROWS_PER_SUPER * SUPERS        # 524,288 rows per core
PAD_ROWS = 0

STEP = 496_512
STARTS = [c * STEP for c in range(N_CORES - 1)] + [BATCH - R]

C_BIAS = 128
C_TOTAL = 135

LAST_RESULTS = None


def _build_consts(ws_logs, bs_logs, ws_b, bs_b):
    import ml_dtypes

    ws_logs = [np.asarray(w, np.float32) for w in ws_logs]
    bs_logs = [np.asarray(b, np.float32) for b in bs_logs]
    ws_b = [np.asarray(w, np.float32) for w in ws_b]
    bs_b = [np.asarray(b, np.float32) for b in bs_b]

    consts = np.zeros((128, C_TOTAL), np.float32)
    consts[:, 0:128] = np.eye(128, dtype=np.float32)
    for k in range(4):
        cat = np.concatenate([bs_logs[k], bs_b[k]])    # [16]
        consts[:, C_BIAS + k] = np.tile(cat, 8)
    consts[:, C_BIAS + 4] = np.concatenate(
        [np.tile(bs_logs[4], 8), np.tile(bs_b[4], 8)]
    )
    consts[:, C_BIAS + 5] = np.tile(bs_logs[4], 16)
    consts[:, C_BIAS + 6] = np.tile(bs_b[4], 16)

    # bf16 stationary matrices, lhsT k at cols [k*128, (k+1)*128)
    wmat = np.zeros((128, 5 * 128), np.float32)
    w1cat = np.vstack([ws_logs[0], ws_b[0]])           # [16, 8]
    for g in range(8):
        wmat[g * 16:g * 16 + 8, g * 16:(g + 1) * 16] = w1cat.T
    for k in (1, 2, 3):
        wk = np.zeros((16, 16), np.float32)
        wk[0:8, 0:8] = ws_logs[k]
        wk[8:16, 8:16] = ws_b[k]
        for g in range(8):
            wmat[g * 16:(g + 1) * 16, k * 128 + g * 16:k * 128 + (g + 1) * 16] = wk.T
    for g in range(8):
        wmat[g * 16:g * 16 + 8, 4 * 128 + g * 8:4 * 128 + (g + 1) * 8] = ws_logs[4].T
        wmat[g * 16 + 8:(g + 1) * 16,
             4 * 128 + 64 + g * 8:4 * 128 + 64 + (g + 1) * 8] = ws_b[4].T
    wmat = np.concatenate([wmat, np.eye(128, dtype=np.float32)], axis=1)
    wmat_bf = wmat.astype(ml_dtypes.bfloat16)
    return consts, wmat_bf


def _ap(t, offset, dims):
    return bass.AP(tensor=t.tensor, offset=t.offset + offset, ap=[t.ap[0]] + dims)


def _build_nc():
    nc = bacc.Bacc()
    z_d = nc.declare_dram_parameter("z", [R, 16], FP, isOutput=False)
    c_d = nc.declare_dram_parameter("consts", [128, C_TOTAL], FP, isOutput=False)
    w_d = nc.declare_dram_parameter("wmat", [128, 6 * 128], BF, isOutput=False)
    o_d = nc.declare_dram_parameter("out", [R, 16], FP, isOutput=True)

    with tile.TileContext(nc) as tc:
        with (
            tc.tile_pool(name="consts", bufs=1) as cp,
            tc.tile_pool(name="nat", bufs=1) as natp,
            tc.tile_pool(name="sb", bufs=1) as sbp,
            tc.tile_pool(name="ps", bufs=1, space="PSUM") as psp,
        ):
            consts = cp.tile([128, C_TOTAL], FP)
            nc.sync.dma_start(out=consts, in_=c_d[:, :])
            wmat = cp.tile([128, 6 * 128], BF)
            nc.sync.dma_start(out=wmat, in_=w_d[:, :])
            identbf = wmat[:, 5 * 128:6 * 128]
            lhsT = [wmat[:, k * 128:(k + 1) * 128] for k in range(5)]
            lhsT5e = lhsT[4][:, 0:64]
            lhsT5b = lhsT[4][:, 64:128]
            biases = [consts[:, C_BIAS + k:C_BIAS + k + 1] for k in range(7)]

            wu1 = sbp.tile([128, 1], FP, tag="wu")
            nc.scalar.copy(out=wu1, in_=biases[0])
            wu2 = sbp.tile([128, 1], FP, tag="wu")
            nc.vector.tensor_copy(out=wu2, in_=biases[0])

            natbfs = {}
            tail_dmas = []

            def load(s):
                if s >= SUPERS:
                    return
                r0 = s * ROWS_PER_SUPER
                natbf = natp.tile([128, 4096], BF, tag="nat", bufs=4)
                nc.gpsimd.dma_start(
                    out=natbf.rearrange("p (c f) -> p c f", c=256, f=16),
                    in_=z_d[r0:r0 + ROWS_PER_SUPER, :].rearrange(
                        "(p c) f -> p c f", p=128, c=256
                    ),
                )
                natbfs[s] = natbf

            def fwdT4(s, j, x0ps):
                """4 transposes: global subtiles 16*half+... j in 0..8 of super s;
                x0ps covers half = j//4 (subtiles u = 16*(j//4) .. +16)."""
                for u in range(j * 4, j * 4 + 4):
                    nc.tensor.transpose(
                        x0ps[:, (u % 16) * 128:(u % 16) * 128 + 128],
                        natbfs[s][:, u * 128:(u + 1) * 128],
                        identbf,
                    )

            def quarter_mms(lhsT_k, h_in, q):
                hps = psp.tile([128, 1024], FP, tag=f"T{q % 2}", bufs=1)
                for n in range(2):
                    src = h_in[:, q * 1024 + n * 512:q * 1024 + (n + 1) * 512]
                    nc.tensor.matmul(hps[:, n * 512:(n + 1) * 512],
                                     lhsT_k, src, start=True, stop=True)
                return hps

            def quarter_prelu(hps, k, hb, q):
                nc.scalar.activation(
                    out=hb[:, q * 1024:(q + 1) * 1024], in_=hps,
                    func=mybir.ActivationFunctionType.Prelu,
                    bias=biases[k], scale=1.0, alpha=0.01,
                )

            def layer1(s, x0):
                hb = sbp.tile([128, 4096], BF, tag="h0", bufs=2)
                for q in range(4):
                    hps = quarter_mms(lhsT[0], x0, q)
                    quarter_prelu(hps, 0, hb, q)
                return hb

            def store(s):
                r0 = s * ROWS_PER_SUPER
                out_dma = nc.gpsimd.dma_start(
                    out=o_d[r0:r0 + ROWS_PER_SUPER, :].rearrange(
                        "(p c) f -> p c f", p=128, c=256
                    ),
                    in_=natbfs[s].rearrange("p (c f) -> p c f", c=256, f=16),
                )
                del natbfs[s]
                load(s + 3)
                if s >= SUPERS - 3:
                    tail_dmas.append(out_dma)

            # ---- prologue: super 0's x0 and L1
            load(0)
            load(1)
            load(2)
            x0 = sbp.tile([128, 4096], BF, tag="x0", bufs=2)
            for half in range(2):
                x0ps = psp.tile([128, 2048], BF, tag="x0ps", bufs=1)
                for j in range(half * 4, half * 4 + 4):
                    fwdT4(0, j, x0ps)
                nc.vector.tensor_copy(
                    out=x0[:, half * 2048:(half + 1) * 2048], in_=x0ps)
            h = layer1(0, x0)

            for s in range(SUPERS):
                nxt = s + 1 < SUPERS
                eb = sbp.tile([128, 4096], BF, tag="eb", bufs=2)
                if nxt:
                    x0n = sbp.tile([128, 4096], BF, tag="x0", bufs=2)
                    x0ps = psp.tile([128, 2048], BF, tag="x0ps", bufs=1)
                # ---- layers 2..4, quarters ping-ponging T0/T1
                for k in (1, 2, 3):
                    hb = sbp.tile([128, 4096], BF, tag=f"h{k}", bufs=2)
                    for q in range(4):
                        hps = quarter_mms(lhsT[k], h, q)
                        if nxt and q in (1, 3):
                            j = (k - 1) * 2 + (q - 1) // 2   # 0..6
                            fwdT4(s + 1, j, x0ps)
                            if j == 3:
                                nc.vector.tensor_copy(
                                    out=x0n[:, 0:2048], in_=x0ps)
                                x0ps = psp.tile([128, 2048], BF,
                                                tag="x0ps", bufs=1)
                        quarter_prelu(hps, k, hb, q)
                    h = hb
                # ---- L5 packed pairs
                for P in range(2):
                    eps = psp.tile([128, 1024], FP, tag="T0", bufs=1)
                    bps = psp.tile([128, 1024], FP, tag="T1", bufs=1)
                    for qq in range(2):
                        q = 2 * P + qq
                        for n in range(2):
                            src = h[:, q * 1024 + n * 512:q * 1024 + (n + 1) * 512]
                            nc.tensor.matmul(
                                eps[64 * qq:64 * qq + 64, n * 512:(n + 1) * 512],
                                lhsT5e, src, start=True, stop=True)
                            nc.tensor.matmul(
                                bps[64 * qq:64 * qq + 64, n * 512:(n + 1) * 512],
                                lhsT5b, src, start=True, stop=True)
                    if nxt and P == 0:
                        fwdT4(s + 1, 6, x0ps)
                        fwdT4(s + 1, 7, x0ps)
                        nc.vector.tensor_copy(out=x0n[:, 2048:4096], in_=x0ps)
                    nc.scalar.activation(
                        out=eb[:, P * 1024:(P + 1) * 1024], in_=eps,
                        func=mybir.ActivationFunctionType.Exp,
                        bias=biases[5], scale=1.0,
                    )
                    nc.vector.tensor_scalar_add(
                        out=eb[:, 2048 + P * 1024:2048 + (P + 1) * 1024],
                        in0=bps, scalar1=biases[6],
                    )
                if nxt:
                    # super s+1's L1 keeps PE busy through the Exp window
                    h = layer1(s + 1, x0n)
                    x0 = x0n

                # ---- backT + combine per pair, then store
                natbf = natbfs[s]
                for P in range(2):
                    eT = psp.tile([128, 1024], BF, tag="eT", bufs=1)
                    bT = psp.tile([128, 1024], BF, tag="bT", bufs=1)
                    for u in range(8):
                        nc.tensor.transpose(
                            eT[:, u * 128:(u + 1) * 128],
                            eb[:, P * 1024 + u * 128:P * 1024 + (u + 1) * 128],
                            identbf,
                        )
                        nc.tensor.transpose(
                            bT[:, u * 128:(u + 1) * 128],
                            eb[:, 2048 + P * 1024 + u * 128:
                                2048 + P * 1024 + (u + 1) * 128],
                            identbf,
                        )
                    for qq in range(2):
                        q = 2 * P + qq
                        e_ap = _ap(eT, 64 * qq, [[128, 8], [8, 8], [1, 8]])
                        b_ap = _ap(bT, 64 * qq, [[128, 8], [8, 8], [1, 8]])
                        zr_ap = _ap(natbf, q * 1024 + 8,
                                    [[128, 8], [16, 8], [1, 8]])
                        tmp = sbp.tile([128, 1024], BF, tag="tmp", bufs=2)
                        tmp_ap = _ap(tmp, 0, [[128, 8], [8, 8], [1, 8]])
                        nc.vector.tensor_mul(out=tmp_ap, in0=e_ap, in1=zr_ap)
                        nc.vector.tensor_add(out=zr_ap, in0=tmp_ap, in1=b_ap)
                store(s)

            flush = sbp.tile([128, 1], FP, tag="wu")
            fl = nc.vector.tensor_copy(out=flush, in_=biases[0])
            for dma in tail_dmas:
                _add_dep_helper(fl.ins, dma.ins, sync=True,
                                reason="drain tail out-DMAs before kernel end")

    nc.finalize()
    return nc


_NC_CACHE = None


def kernel(z, ws_logs, bs_logs, ws_b, bs_b):
    global _NC_CACHE, LAST_RESULTS
    z = np.asarray(z, np.float32)
    assert z.shape == (BATCH, 16)
    consts, wmat_bf = _build_consts(ws_logs, bs_logs, ws_b, bs_b)

    if _NC_CACHE is None:
        _NC_CACHE = _build_nc()
    nc = _NC_CACHE

    in_maps = []
    for s in STARTS:
        in_maps.append({"z": np.ascontiguousarray(z[s:s + R]),
                        "consts": consts, "wmat": wmat_bf})
    trace = bool(os.environ.get("AFFINE_TRACE"))
    res = run_bass_kernel_spmd(nc, in_maps, core_ids=list(range(N_CORES)), trace=trace)
    LAST_RESULTS = res

    out = np.empty((BATCH, 16), np.float32)
    for c in range(N_CORES):
        out[STARTS[c]:STARTS[c] + R] = res.results[c]["out"]
    return out
